# revision 9
# baseline (speedup 1.0000x reference)
"""Fused int8 dequant -> causal mask -> softmax -> int8 requant on 8 TRN2 cores.

Problem: x_q [B=4, H=16, S=1024, S] int8, per-(head,row) scales sx/so [H*S] f32.
  out = int8(clip(round(softmax(causal_mask(x_q * sx)) / so), -128, 127))

Sharding: 2 heads per core (data parallel over the 64 independent (b, h)
planes; grouping by head lets the 4 batches of one head share per-partition
scale vectors).

Per-core algorithm, rows on partitions, softmax along the free dim. For each
(h, t) row-tile of 128 rows only cols [0, W=(t+1)*128) can be nonzero
(causal), so only those are loaded/stored (44% of DMA traffic saved; the
untouched upper triangle stays zero because output buffers are pre-zeroed by
the runtime). The causal mask inside the loaded region affects only the
diagonal 128x128 block and is handled without any masked reduce:

  host: x is pre-masked (strict upper triangle zeroed) -> masked exp gives
        exp(0) = 1, so each row's sum is over-counted by exactly (127 - p)
        ones (p = partition), a compile-time constant vector.
    1. one DMA loads [128, 4b, W] int8
    2. ScalarE per b: e = Exp(sx[row] * x) from int8 (scale per-partition),
       accum_out -> row sums for free
    3. smalls: sum_v = sum - corr;  r = 1/(sum_v*so)
    4. DVE per b: y = e*r cast to int8 -- the HW f32->int8 conversion is
       round-to-nearest-even WITH saturation (measured on both DVE and ACT),
       which is exactly jnp's round+clip
    5. masked positions produced y = round(r) != 0 -> zero them by an
       in-place int8 multiply of the diagonal block with a lower-tri 0/1 mask
    6. one DMA stores [128, 4b, W] int8
"""

import contextlib
import ctypes
import os
import sys
import types
from contextlib import ExitStack

import numpy as np

import concourse.bacc as bacc
import concourse.bass as bass
import concourse.tile as tile
from concourse import mybir
from concourse.bass_utils import run_bass_kernel_spmd

B, H, S = 4, 16, 1024
NCORES = 8
HPC = H // NCORES  # heads per core
P = 128
NT = S // P  # row tiles per plane
AF = mybir.ActivationFunctionType
ALU = mybir.AluOpType

_AXON_SO = "/opt/axon/libaxon_pjrt.so"


def _ensure_ntff_hook():
    """This image's antenv lacks axon_hooks; provide it so trace=True works."""
    if "antenv.axon_hooks" in sys.modules:
        return
    import antenv

    mod = types.ModuleType("antenv.axon_hooks")
    state = {"hook": None}
    mod.set_axon_ntff_profile_hook = lambda h: state.__setitem__("hook", h)
    mod.get_axon_ntff_profile_hook = lambda: state["hook"]
    sys.modules["antenv.axon_hooks"] = mod
    antenv.axon_hooks = mod

    if not os.path.exists(_AXON_SO):
        return
    lib = ctypes.CDLL(_AXON_SO)
    if not hasattr(lib, "axon_start_nrt_profile"):
        return
    lib.axon_start_nrt_profile.argtypes = [ctypes.POINTER(ctypes.c_int64), ctypes.c_size_t]
    lib.axon_start_nrt_profile.restype = ctypes.c_int64
    lib.axon_stop_nrt_profile.argtypes = [ctypes.c_char_p]
    lib.axon_stop_nrt_profile.restype = ctypes.c_int64

    @contextlib.contextmanager
    def _hook(output_dir, device_ids):
        import jax

        jax.devices()
        if device_ids:
            ids = (ctypes.c_int64 * len(device_ids))(*device_ids)
            rc = lib.axon_start_nrt_profile(ids, len(device_ids))
        else:
            rc = lib.axon_start_nrt_profile(None, 0)
        if rc != 0:
            raise RuntimeError(f"axon_start_nrt_profile rc={rc}")
        try:
            yield
        finally:
            n = lib.axon_stop_nrt_profile(str(output_dir).encode())
            print(f"profile: {n} file(s) written to {output_dir}", file=sys.stderr)

    mod.set_axon_ntff_profile_hook(_hook)


_cached_nc = None


def _build_bass(compile=True):
    nc = bacc.Bacc("TRN2", target_bir_lowering=False, debug=False,
                   num_devices=NCORES)
    x = nc.declare_dram_parameter("x", [HPC, B, S, S], mybir.dt.int8, isOutput=False)
    sx = nc.declare_dram_parameter("sx", [P, HPC * NT], mybir.dt.float32, isOutput=False)
    so = nc.declare_dram_parameter("so", [P, HPC * NT], mybir.dt.float32, isOutput=False)
    corr = nc.declare_dram_parameter("corr", [P, 1], mybir.dt.float32, isOutput=False)
    tri = nc.declare_dram_parameter("tri", [P, P], mybir.dt.int8, isOutput=False)
    y = nc.declare_dram_parameter("y", [HPC, B, S, S], mybir.dt.int8, isOutput=True)

    with ExitStack() as ctx:
        tc = ctx.enter_context(tile.TileContext(nc))
        singles = ctx.enter_context(tc.tile_pool(name="singles", bufs=1))
        xpool = ctx.enter_context(tc.tile_pool(name="xp", bufs=3))
        epool = ctx.enter_context(tc.tile_pool(name="ep", bufs=2))
        ypool = ctx.enter_context(tc.tile_pool(name="yp", bufs=3))
        smalls = ctx.enter_context(tc.tile_pool(name="sm", bufs=4))

        sxt = singles.tile([P, HPC * NT], mybir.dt.float32)
        nc.sync.dma_start(sxt[:], sx[:])
        sot = singles.tile([P, HPC * NT], mybir.dt.float32)
        nc.sync.dma_start(sot[:], so[:])
        corrt = singles.tile([P, 1], mybir.dt.float32)
        nc.sync.dma_start(corrt[:], corr[:])
        trit = singles.tile([P, P], mybir.dt.int8)
        nc.sync.dma_start(trit[:], tri[:])

        for h in range(HPC):
            for t in range(NT):
                W = (t + 1) * P
                col = h * NT + t

                xt = xpool.tile([P, B, W], mybir.dt.int8, tag="xt")
                nc.sync.dma_start(
                    xt[:], x[h, :, t * P:(t + 1) * P, 0:W].rearrange("b r c -> r b c")
                )

                et = epool.tile([P, B, W], mybir.dt.float32, tag="et")
                sums = smalls.tile([P, B], mybir.dt.float32, tag="sums")
                for b in range(B):
                    nc.scalar.activation(et[:, b, :], xt[:, b, :], AF.Exp,
                                         bias=0.0, scale=sxt[:, col:col + 1],
                                         accum_out=sums[:, b:b + 1])

                rt = smalls.tile([P, B], mybir.dt.float32, tag="rt")
                nc.vector.tensor_scalar(rt[:], sums[:], corrt[:], None, ALU.subtract)
                nc.vector.tensor_scalar(rt[:], rt[:], sot[:, col:col + 1], None, ALU.mult)
                nc.vector.reciprocal(rt[:], rt[:])

                yt = ypool.tile([P, B, W], mybir.dt.int8, tag="yt")
                for b in range(B):
                    nc.vector.tensor_scalar(yt[:, b, :], et[:, b, :],
                                            rt[:, b:b + 1], None, ALU.mult)
                    nc.vector.tensor_tensor(yt[:, b, t * P:(t + 1) * P],
                                            yt[:, b, t * P:(t + 1) * P],
                                            trit[:], ALU.mult)

                nc.sync.dma_start(
                    y[h, :, t * P:(t + 1) * P, 0:W].rearrange("b r c -> r b c"), yt[:]
                )
    if compile:
        nc.compile()
    return nc


_tril_mask = None


def _host_prep(x_q, scale_x, scale_out):
    global _tril_mask
    x_q = np.asarray(x_q)
    assert x_q.dtype == np.int8, x_q.dtype
    scale_x = np.asarray(scale_x, dtype=np.float32).reshape(H, S)
    scale_out = np.asarray(scale_out, dtype=np.float32).reshape(H, S)

    if _tril_mask is None:
        _tril_mask = np.tril(np.ones((S, S), dtype=np.int8))
    x_pm = x_q * _tril_mask  # zero the strict upper triangle (broadcasts over B, H)

    # [P, H, NT]: sxr[p, h, t] = scale_x[h, t*128 + p]
    sxr = scale_x.reshape(H, NT, P).transpose(2, 0, 1)
    sor = scale_out.reshape(H, NT, P).transpose(2, 0, 1)

    p = np.arange(P)
    corr = (127 - p).astype(np.float32).reshape(P, 1)
    tri = np.tril(np.ones((P, P), dtype=np.int8))

    in_maps = []
    for c in range(NCORES):
        hs = slice(c * HPC, (c + 1) * HPC)
        xc = np.ascontiguousarray(x_pm[:, hs].transpose(1, 0, 2, 3))
        sxc = np.ascontiguousarray(sxr[:, hs].reshape(P, HPC * NT))
        soc = np.ascontiguousarray(sor[:, hs].reshape(P, HPC * NT))
        in_maps.append({"x": xc, "sx": sxc, "so": soc, "corr": corr, "tri": tri})
    return in_maps


def run(x_q, scale_x, scale_out, trace=False):
    global _cached_nc
    if trace:
        _ensure_ntff_hook()
    if _cached_nc is None:
        _cached_nc = _build_bass()
    in_maps = _host_prep(x_q, scale_x, scale_out)
    res = run_bass_kernel_spmd(_cached_nc, in_maps, core_ids=list(range(NCORES)),
                               trace=trace)
    out = np.empty((B, H, S, S), np.int8)
    for c in range(NCORES):
        yc = np.asarray(res.results[c]["y"])
        out[:, c * HPC:(c + 1) * HPC] = yc.transpose(1, 0, 2, 3)
    return out, res


def kernel(x_q, scale_x, scale_out):
    out, _ = run(x_q, scale_x, scale_out,
                 trace=bool(int(os.environ.get("KERNEL_TRACE", "0"))))
    return out
